# revision 39
# baseline (speedup 1.0000x reference)
"""ConvNAT (conv QKV + 2D dilated neighborhood attention) on 8 trn2 cores.

Sharding: core = (batch b, H-slab of 24 rows).  Each core computes conv
q/k/v for the 36 kv rows its attention actually reads (image rows
h0-6 .. h0+29) and the attention for its 24 output rows.  H-border rows
(h<6, h>=90), whose NATTEN windows are clamped, are computed on the host
and overwrite the device output.

v2 notes (PE p-state aware schedule):
  * x is loaded as 6 element-shifted flat copies of the same contiguous
    per-partition HBM run (shifts 0/98/1/99 give all row/col tap
    alignments) -- every DMA is one big descriptor per partition.
  * conv is emitted group-major so each psum bank recycles early
    (bufs=4) and the PE stream has no intra-wave stalls.
  * the logits prologue (key rows 0..12) is interleaved INTO the v-conv
    phase so the PE never idles across the conv->attention transition;
    idle gaps reset the tensor engine's DVFS ramp (0.42 -> 0.83 ns/col).
  * W-position bias+mask is applied multiplicatively AFTER exp
    (exp(L+wb) == exp(L)*exp(wb), exp(NEG)=0 masks exactly): ACT does
    exp directly out of PSUM, DVE does a cheap bf16 2X/4X multiply, and
    the psO evacuation moves to Pool.  Per-step engine loads stay under
    the PE's full-speed step time, keeping the PE gapless (and ramped).

Attention (transposed-logits form), unchanged math:
  L^T[kc,(j,q)] = K_ext_r^T Q_ext ; Q/K_ext = [conv ch | 7 h-dist ch].
  px = exp(L^T) * exp(wb)  (wb = scale*pe_w.pe_w + NEG mask, host table)
  psum[96q,129] = sum_i px_i^T^T vt_aug[r_i], col 128 = softmax denom,
  Pool multiplies by 1/den during psum evacuation.
"""
import os
import re
import sys

sys.path.insert(0, '/opt/trn_rl_repo')

import numpy as np
import ml_dtypes

import concourse.bass as bass
from concourse import mybir
from concourse.tile import TileContext
from concourse.masks import make_identity
from bass_rust import ScopedClock, VectorClock

F32 = mybir.dt.float32
BF16 = mybir.dt.bfloat16
NPBF16 = ml_dtypes.bfloat16

B, CIN, H, W = 2, 64, 96, 96
CI, CO = 64, 128
KS, DIL = 7, 2
SCALE = float(CI * 2) ** -0.5  # Cqk = 128 after pe concat
HS = 24          # rows per core
NH = 4           # h-slabs
NCORES = 8
NDIST = 7        # h-distance channels
CH = 64 + NDIST  # contraction channels
KV = 36          # kv rows per core (24 + 6 halo each side; row r = img h0-6+r)
XR = 38          # x slab rows (KV + conv halo; row t = img h0-7+t)
XC = 98          # x slab cols (96 + pad)
QOFF = 6         # q row j = kv row j+6
NEG = -30000.0
RING = 23        # px ring depth (av(j) reads key rows j..j+12; logits runs ~21 ahead)
AVW = 130        # AV moving width: 128 v + ones col + pad (even for bf16)
VTW = 136        # vt row stride (128 v + 1 ones + pad)

GRP = ((0, 5), (5, 10), (10, 15), (15, 20),
       (20, 24), (24, 28), (28, 32), (32, 36))
# logits prologue rows emitted after each v-conv group (front-loaded so the
# exps of the last rows clear the ACT queue before the bridge)
PROLOG = ((0, 1, 2), (3, 4, 5), (6, 7), (8, 9), (10, 11), (12, 13), (14,), (15,))
AHEAD = 18                     # steady pair jp: logits(2jp+AHEAD+2, +3)

# ---------------------------------------------------------------- compat ---
MAX_WAITS = 1


def _patched_drain(self, tick_clock, wait_clock):
    nc = self.nc
    ticks = [int(v) for v in re.findall(r'\d+', repr(tick_clock.global_clock))]
    for i in range(0, len(ticks), MAX_WAITS):
        chunk = [0] * len(ticks)
        chunk[i:i + MAX_WAITS] = ticks[i:i + MAX_WAITS]
        if any(chunk):
            probe = nc.sync.nop()
            wait_clock.add_sem_waits(probe.ins, ScopedClock({None: VectorClock(chunk)}))
    nc.sync.drain()
    nc.all_engine_barrier()
    popped = nc._tile_sem_poison_stack.pop()
    assert popped is self._sem_poison
    nc.clear_and_free_semaphores(list(self.sems.allocated().values()))
    nc.all_engine_barrier()


TileContext._drain_and_barrier = _patched_drain


def _split_excess_waits(nc, max_waits=MAX_WAITS):
    n_split = 0
    for fn in nc.m.functions:
        for bb in fn.blocks:
            out = []
            changed = False
            for inst in bb.instructions:
                si = inst.sync_info
                waits = list(si.on_wait) if si and si.on_wait else []
                if len(waits) > max_waits:
                    extra = waits[:-max_waits]
                    for j in range(0, len(extra), max_waits):
                        nop = mybir.InstNoOp(name=f"{inst.name}-ws{j}", ins=[], outs=[])
                        nop.engine = inst.engine
                        nop.sync_info = mybir.SyncInfo(
                            on_wait=extra[j:j + max_waits], on_update=[])
                        out.append(nop)
                    si.on_wait = waits[-max_waits:]
                    changed = True
                    n_split += 1
                out.append(inst)
            if changed:
                bb.instructions = out
    return n_split


# ------------------------------------------------------------- host math ---
def _sincos(length, dim):
    half = dim // 2
    inv_freq = 1.0 / (10000.0 ** (np.arange(half, dtype=np.float64) * 2.0 / dim))
    ang = np.arange(length, dtype=np.float64)[:, None] * inv_freq[None, :]
    return np.concatenate([np.sin(ang), np.cos(ang)], axis=-1)  # (L, dim)


def _na_indices(L, K, D):
    i = np.arange(L)
    g = i % D
    r = i // D
    Lg = (L - g + D - 1) // D
    start = np.clip(r - K // 2, 0, Lg - K)
    return g[:, None] + (start[:, None] + np.arange(K)[None, :]) * D  # (L, K)


def _hdist_channels():
    """QD,KD (NDIST, 96): sum_m QD[m,h]*KD[m,h'] == SCALE*pe_h[h].pe_h[h']
    exactly for even |h-h'| <= 6."""
    pe = _sincos(H, 32)           # (96, 32)
    inv_freq = 1.0 / (10000.0 ** (np.arange(16, dtype=np.float64) * 2.0 / 32))
    dv = np.array([0., 2., 4., 6.])
    g = SCALE * np.cos(dv[:, None] * inv_freq[None, :]).sum(1)  # exact pe.pe(d)
    th = np.arange(4, dtype=np.float64) * (np.pi / 6.0)
    M = np.cos(dv[:, None] * th[None, :])                        # (4, 4)
    b = np.linalg.solve(M, g)
    hh = np.arange(H, dtype=np.float64)
    QD = np.zeros((NDIST, H))
    KD = np.zeros((NDIST, H))
    QD[0] = b[0]
    KD[0] = 1.0
    for m in range(1, 4):
        QD[2 * m - 1] = b[m] * np.cos(th[m] * hh)
        QD[2 * m] = b[m] * np.sin(th[m] * hh)
        KD[2 * m - 1] = np.cos(th[m] * hh)
        KD[2 * m] = np.sin(th[m] * hh)
    # verify
    got = QD.T @ KD
    ref = SCALE * (pe @ pe.T)
    for dd in (-6, -4, -2, 0, 2, 4, 6):
        idx = np.arange(max(0, -dd), min(H, H - dd))
        err = np.abs(got[idx, idx + dd] - ref[idx, idx + dd]).max()
        assert err < 1e-6, (dd, err)
    return QD.astype(np.float32), KD.astype(np.float32)


def _wbias():
    pe = _sincos(W, 32)
    idx_w = _na_indices(W, KS, DIL)   # (96, 7)
    wb = np.full((W, W), NEG, dtype=np.float64)
    dot = SCALE * (pe @ pe.T)
    for w in range(W):
        wb[w, idx_w[w]] = dot[w, idx_w[w]]
    return wb


def _conv_np(x, w, bias, rows):
    """NCHW 3x3 pad-1 conv evaluated at `rows` -> (B, len(rows), 96, Cout)."""
    Bn, Cin, Hn, Wn = x.shape
    xp = np.zeros((Bn, Cin, Hn + 2, Wn + 2), dtype=np.float64)
    xp[:, :, 1:-1, 1:-1] = x
    rows = np.asarray(rows)
    acc = np.zeros((Bn, len(rows), Wn, w.shape[0]), dtype=np.float64)
    for ky in range(3):
        for kx in range(3):
            xs = xp[:, :, rows + ky, :][:, :, :, kx:kx + Wn]  # (B,C,R,W)
            acc += np.einsum('bcrw,oc->brwo', xs, w[:, :, ky, kx].astype(np.float64))
    return acc + bias[None, None, None, :].astype(np.float64)


def _host_border(x, wq, bq, wk, bk, wv, bv):
    """Reference computation for the clamped border rows. -> dict h -> (B,96,128)."""
    border_h = list(range(0, 6)) + list(range(90, 96))
    kv_rows = sorted(set(np.concatenate([_na_indices(H, KS, DIL)[h] for h in border_h])))
    kv_rows = np.asarray(kv_rows)
    q_c = _conv_np(x, wq, bq, np.asarray(border_h))     # (B, 12, 96, 64)
    k_c = _conv_np(x, wk, bk, kv_rows)                  # (B, R, 96, 64)
    v_c = _conv_np(x, wv, bv, kv_rows)                  # (B, R, 96, 128)
    kv_pos = {r: i for i, r in enumerate(kv_rows)}
    pe_h = _sincos(H, 32)
    pe_w = _sincos(W, 32)
    idx_h = _na_indices(H, KS, DIL)
    idx_w = _na_indices(W, KS, DIL)
    out = {}
    for bi, h in enumerate(border_h):
        pe_q = np.concatenate([np.repeat(pe_h[h][None], W, 0), pe_w], axis=1)  # (96,64)
        q = np.concatenate([q_c[:, bi], np.repeat(pe_q[None], B, 0)], axis=2)  # (B,96,128)
        rows = [kv_pos[r] for r in idx_h[h]]
        kk = k_c[:, rows]                                   # (B,7,96,64)
        vv = v_c[:, rows]                                   # (B,7,96,128)
        pe_k = np.concatenate(
            [np.repeat(pe_h[idx_h[h]][:, None, :], W, 1),
             np.repeat(pe_w[None], KS, 0)], axis=2)         # (7,96,64)
        kk = np.concatenate([kk, np.repeat(pe_k[None], B, 0)], axis=3)  # (B,7,96,128)
        kn = kk[:, :, idx_w]                                # (B,7,96,7,128)
        vn = vv[:, :, idx_w]
        logits = SCALE * np.einsum('bwc,biwjc->bwij', q, kn)   # (B,96,7,7)
        m = logits.reshape(B, W, -1).max(-1)
        p = np.exp(logits - m[:, :, None, None])
        p /= p.reshape(B, W, -1).sum(-1)[:, :, None, None]
        out[h] = np.einsum('bwij,biwjc->bwc', p, vn)        # (B,96,128)
    return out


def _users(r):
    """Query rows j (0..23) attending key row r: j = r-2i, i=0..6."""
    j0 = r - 12 if r >= 12 else r % 2
    j1 = min(r, 23)
    if j0 > j1:
        return j0, 0
    return j0, (j1 - j0) // 2 + 1


# ------------------------------------------------------------ bass build ---
_CACHE = {}


def _build_program():
    if 'nc' in _CACHE:
        return _CACHE['nc']
    nc = bass.Bass('TRN2')
    xs = nc.dram_tensor('xs', (64, XR, XC), BF16, kind='ExternalInput')
    # weights packed into one tensor per conv layer (HWDGE serializes DMAs
    # per queue, so fewer/bigger transfers win):
    #   cols 0:384 = ky01 taps (3x128 contract-pairs), 384:512 = ky2 colpair,
    #   rows 0:64 cols 512:640 = ky2 kx2 (64-contract)
    wqk = nc.dram_tensor('wqk', (128, 640), BF16, kind='ExternalInput')
    wv = nc.dram_tensor('wv', (128, 640), BF16, kind='ExternalInput')
    qvb = nc.dram_tensor('qvb', (128, 2), F32, kind='ExternalInput')
    qd = nc.dram_tensor('qd', (NDIST, HS * 96), BF16, kind='ExternalInput')
    kd = nc.dram_tensor('kd', (NDIST, KV * 96), BF16, kind='ExternalInput')
    ewb = nc.dram_tensor('ewb', (96, 96), BF16, kind='ExternalInput')
    # (96, HS, 129): w-major so a 2-row store is one descriptor/partition;
    # col 128 carries the softmax denominator; host transposes + divides
    o = nc.dram_tensor('o', (96, HS, 129), F32, kind='ExternalOutput')

    with TileContext(nc) as tc:
        with tc.tile_pool(name='persist', bufs=1) as pp:
            # ---- x: three element-shifted flat HBM streams (pitch XC):
            #   x2  bot = shift 0 (row t, col c)   top = shift 98 (row t+1)
            #   xw  top = shift 1 (row t, col c+1)
            # derived on otherwise-idle engines:
            #   x2b[0:128] = x2 cols 1:97  (DVE col-shift copy, both halves)
            #   xw[0:64]   = x2 bot cols 0:96  (ACT copy; Pool copies are 3x
            #                slower and SWDGE triggers hog the Pool queue)
            # HWDGE serializes DMAs per queue (~1.7us each), so x rides on
            # all three queues: x2 bot on sync, x2 top on scalar, xw top on
            # gpsimd, chunked (0,7)/(7,22)/(22,38) to bound the first-tap
            # latency while keeping the queue depth small.
            x2 = pp.tile([128, XR, XC], BF16)
            x2b = pp.tile([128, XR, 96], BF16)
            xw = pp.tile([128, XR, XC], BF16)
            xs_ap = xs[:]

            def flat_dma(eng, tile, p0, a, b, shift):
                off = a * XC + shift
                n = min((b - a) * XC, XR * XC - off)
                if n <= 0:
                    return
                src = bass.AP(tensor=xs_ap.tensor, offset=xs_ap.offset + off,
                              ap=[xs_ap.ap[0], [1, n]])
                d0 = tile[p0:p0 + 64, a:b, :]
                dst = bass.AP(tensor=d0.tensor, offset=d0.offset,
                              ap=[d0.ap[0], [1, n]])
                eng.dma_start(out=dst, in_=src)

            def xchunk(a, b):
                flat_dma(nc.sync, x2, 0, a, b, 0)
                flat_dma(nc.scalar, x2, 64, a, b, XC)
                flat_dma(nc.gpsimd, xw, 64, a, b, 1)
                b37 = min(b, XR - 1)
                nc.vector.tensor_copy(out=x2b[:, a:b37, :],
                                      in_=x2[:, a:b37, 1:97])
                if b > b37:
                    nc.vector.tensor_copy(out=x2b[0:64, b37:b, :],
                                          in_=x2[0:64, b37:b, 1:97])
                nc.scalar.copy(out=xw[0:64, a:b, 0:96],
                               in_=x2[0:64, a:b, 0:96])

            xchunk(0, 7)
            # qk weights: one packed DMA on the gpsimd queue (keeps the sync
            # queue clear for the x2 chunks the first conv groups gate on)
            w_all = pp.tile([128, 640], BF16)
            nc.gpsimd.dma_start(out=w_all, in_=wqk[:])
            xchunk(7, 22)
            xchunk(22, XR)
            qvbias = pp.tile([128, 2], F32)
            nc.scalar.dma_start(out=qvbias, in_=qvb[:])

            # ---- v weights + tables (needed later; queue behind x)
            wv_all = pp.tile([128, 640], BF16)
            nc.scalar.dma_start(out=wv_all, in_=wv[:])
            ewb_sb = pp.tile([96, 96], BF16)
            nc.scalar.dma_start(out=ewb_sb, in_=ewb[:])
            ident = pp.tile([128, 128], BF16)
            make_identity(nc, ident)

            q_ext = pp.tile([CH, HS * 96], BF16)
            k_ext = pp.tile([CH, KV * 96], BF16)
            vsb = pp.tile([128, KV * 96], BF16)
            vt = pp.tile([96, KV, VTW], BF16)
            nc.sync.dma_start(out=q_ext[64:CH, :], in_=qd[:])
            nc.gpsimd.dma_start(out=k_ext[64:CH, :], in_=kd[:])
            nc.gpsimd.memset(vt[:, :, 128:130], 1.0)

            with tc.tile_pool(name='lps', bufs=2, space='PSUM') as lps, \
                 tc.tile_pool(name='pxr', bufs=3) as pxr, \
                 tc.tile_pool(name='pxp', bufs=RING) as pxp, \
                 tc.tile_pool(name='att', bufs=3) as att:
                pexp = {}
                ewb_ap = ewb_sb[:, :]

                def logits(r):
                    j0, n = _users(r)
                    # rows padded to 128 f32 so 4 rows = one PSUM bank
                    psL = lps.tile([96, 7, 128], F32, tag='L')
                    ke = k_ext[:, r * 96:(r + 1) * 96]
                    for (u0, u1) in ((0, min(n, 4)), (4, n)):
                        if u0 >= u1:
                            continue
                        qa = q_ext[:, (j0 + 2 * u0) * 96:(j0 + 2 * u0 + 1) * 96]
                        rhs = bass.AP(tensor=qa.tensor, offset=qa.offset,
                                      ap=[qa.ap[0], [2 * 96, u1 - u0], qa.ap[1]])
                        nc.tensor.matmul(psL[:, u0:u1, 0:96], ke, rhs,
                                         start=True, stop=True)
                    # exp straight out of psum; W-bias applied multiplicatively
                    # (exp(NEG)=0 masks exactly), bf16 all-SBUF 2X on DVE.
                    pxr_t = pxr.tile([96, 7, 96], BF16, tag='pxr')
                    nc.scalar.activation(out=pxr_t[:, 0:n, :],
                                         in_=psL[:, 0:n, 0:96],
                                         func=mybir.ActivationFunctionType.Exp)
                    px = pxp.tile([96, 7, 96], BF16, tag='px')
                    ewb_b = bass.AP(tensor=ewb_ap.tensor, offset=ewb_ap.offset,
                                    ap=[ewb_ap.ap[0], [0, n], ewb_ap.ap[1]])
                    # all-SBUF bf16 multiply; Pool (otherwise idle) takes the
                    # late rows fully and the tail slots of the early steady
                    # rows, so DVE keeps up with oh/vt evacuations
                    if r >= 24 or r in (14, 15):
                        nc.gpsimd.tensor_tensor(out=px[:, 0:n, :],
                                                in0=pxr_t[:, 0:n, :],
                                                in1=ewb_b,
                                                op=mybir.AluOpType.mult)
                    elif r >= 18 and n > 5:
                        ewb_b5 = bass.AP(tensor=ewb_ap.tensor,
                                         offset=ewb_ap.offset,
                                         ap=[ewb_ap.ap[0], [0, 5],
                                             ewb_ap.ap[1]])
                        ewb_b2 = bass.AP(tensor=ewb_ap.tensor,
                                         offset=ewb_ap.offset,
                                         ap=[ewb_ap.ap[0], [0, n - 5],
                                             ewb_ap.ap[1]])
                        nc.vector.tensor_tensor(out=px[:, 0:5, :],
                                                in0=pxr_t[:, 0:5, :],
                                                in1=ewb_b5,
                                                op=mybir.AluOpType.mult)
                        nc.gpsimd.tensor_tensor(out=px[:, 5:n, :],
                                                in0=pxr_t[:, 5:n, :],
                                                in1=ewb_b2,
                                                op=mybir.AluOpType.mult)
                    else:
                        nc.vector.tensor_tensor(out=px[:, 0:n, :],
                                                in0=pxr_t[:, 0:n, :],
                                                in1=ewb_b,
                                                op=mybir.AluOpType.mult)
                    pexp[r] = px

                # ---------------------------------------------- convolution --
                # group-major: each group's 5 taps back-to-back, psum bank
                # recycles as soon as its evac lands (bufs=4).  The logits
                # prologue (rows 0..12) interleaves into the v layer so the
                # PE stream never breaks at the conv->attention transition.
                with tc.tile_pool(name='cps', bufs=4, space='PSUM') as cps:
                    for which in range(2):  # 0 = qk, 1 = v
                        wt = w_all if which == 0 else wv_all
                        for g in range(8):
                            r0, r1 = GRP[g]
                            sz = r1 - r0
                            ps = cps.tile([128, 5, 96], F32, tag='c',
                                          name=f'c{which}_{g}')
                            nc.tensor.matmul(ps[:, 0:sz, :], wt[:, 0:128],
                                             x2[:, r0:r1, 0:96],
                                             start=True, stop=False)
                            nc.tensor.matmul(ps[:, 0:sz, :], wt[:, 128:256],
                                             x2b[:, r0:r1, 0:96],
                                             start=False, stop=False)
                            nc.tensor.matmul(ps[:, 0:sz, :], wt[:, 256:384],
                                             x2[:, r0:r1, 2:XC],
                                             start=False, stop=False)
                            nc.tensor.matmul(ps[:, 0:sz, :], wt[:, 384:512],
                                             xw[:, r0 + 2:r1 + 2, 0:96],
                                             start=False, stop=False)
                            nc.tensor.matmul(ps[:, 0:sz, :],
                                             wt[0:64, 512:640],
                                             x2[0:64, r0 + 2:r1 + 2, 2:XC],
                                             start=False, stop=True)
                            if which == 0:
                                # k rows: all; q rows: kv rows 6..29 only
                                nc.vector.tensor_scalar(
                                    out=k_ext[0:64, r0 * 96:r1 * 96],
                                    in0=ps[64:128, 0:sz, :],
                                    scalar1=qvbias[64:128, 0:1],
                                    scalar2=None, op0=mybir.AluOpType.add)
                                qa = max(r0, QOFF) - r0
                                qb = min(r1, QOFF + HS) - r0
                                if qa < qb:
                                    nc.scalar.activation(
                                        out=q_ext[0:64, (r0 + qa - QOFF) * 96:
                                                  (r0 + qb - QOFF) * 96],
                                        in_=ps[0:64, qa:qb, :],
                                        func=mybir.ActivationFunctionType.Identity,
                                        bias=qvbias[0:64, 0:1])
                            else:
                                if g % 2 == 0:
                                    nc.scalar.activation(
                                        out=vsb[:, r0 * 96:r1 * 96],
                                        in_=ps[:, 0:sz, :],
                                        func=mybir.ActivationFunctionType.Identity,
                                        bias=qvbias[:, 1:2])
                                else:
                                    nc.vector.tensor_scalar(
                                        out=vsb[:, r0 * 96:r1 * 96],
                                        in0=ps[:, 0:sz, :],
                                        scalar1=qvbias[:, 1:2],
                                        scalar2=None, op0=mybir.AluOpType.add)
                                for r in PROLOG[g]:
                                    logits(r)

                # ------------------------------- V^T + attention steady ----
                with tc.tile_pool(name='tps', bufs=1, space='PSUM') as tps, \
                     tc.tile_pool(name='ops', bufs=3, space='PSUM') as ops:
                    # NOTE: a start=True matmul marks its whole 2KB psum bank
                    # pending-zero, so two 520B psO slots share one bank: the
                    # second accumulation group opens with start=False and its
                    # first write lands on pending-zero bytes (write mode).
                    dve_cp = nc.vector.tensor_copy
                    act_cp = nc.scalar.copy
                    vt_pend = []

                    def vtrans(grp, enga=None, engb=None):
                        pst = tps.tile([96, 4, 128], BF16, tag='t')
                        for rr in range(4):
                            r = grp * 4 + rr
                            nc.tensor.transpose(
                                pst[:, rr, :], vsb[:, r * 96:(r + 1) * 96], ident)
                        if enga is None:
                            vt_pend.extend((pst, grp, rr) for rr in range(4))
                        else:
                            enga(out=vt[:, grp * 4:grp * 4 + 2, 0:128],
                                 in_=pst[:, 0:2, :])
                            engb(out=vt[:, grp * 4 + 2:grp * 4 + 4, 0:128],
                                 in_=pst[:, 2:4, :])

                    def av_pair(j0):
                        psO = ops.tile([96, 2, 136], F32, tag='O')
                        for s in range(2):
                            j = j0 + s
                            for i in range(KS):
                                r = j + 2 * i
                                ju, _ = _users(r)
                                slot = (j - ju) // 2
                                nc.tensor.matmul(psO[:, s, 0:AVW],
                                                 pexp[r][:, slot, :],
                                                 vt[:, r, 0:AVW],
                                                 start=(s == 0 and i == 0),
                                                 stop=(i == KS - 1),
                                                 skip_group_check=(s == 1))
                        # raw evac incl. denominator col; host divides
                        oh = att.tile([96, 2, 129], F32, tag='oh')
                        nc.vector.tensor_copy(out=oh[:], in_=psO[:, 0:2, 0:129])
                        nc.sync.dma_start(out=o[:, j0:j0 + 2, :], in_=oh[:])

                    # bridge: v^T transposes while the conv psum pool drains;
                    # logits interleave to cover the tps=1 evac latency.
                    # DVE takes most evac halves -- ACT must keep exps flowing
                    # or the first steady logits stall (p-state demotion).
                    for grp in range(6):
                        if grp % 2:
                            vtrans(grp, dve_cp, act_cp)
                        else:
                            vtrans(grp, act_cp, dve_cp)
                        if grp < 4:
                            logits(AHEAD - 2 + grp)        # rows 16..19
                    for jp in range(HS // 2):
                        j0 = 2 * jp
                        # av first: its ~1.1us of PE work gives the ACT exp
                        # queue slack before the psL-reuse wait below
                        av_pair(j0)
                        for r in (j0 + AHEAD + 2, j0 + AHEAD + 3):
                            if r < KV:
                                logits(r)
                        if jp in (1, 3, 5):
                            vtrans(6 + jp // 2)     # evacs drain below
                        for _ in range(2):
                            if vt_pend:
                                pst, grp, rr = vt_pend.pop(0)
                                dve_cp(out=vt[:, grp * 4 + rr, 0:128],
                                       in_=pst[:, rr, :])

    _split_excess_waits(nc)
    _CACHE['nc'] = nc
    return nc


# ---------------------------------------------------------------- kernel ---
def _pack_layer(wa, wb):
    """Pack a conv layer's taps into the (128, 640) device layout.
    wa/wb: (64-or-128 out-ch halves already merged) -> see caller."""
    out = np.zeros((128, 640), dtype=np.float32)
    for kx in range(3):
        out[:, kx * 128:(kx + 1) * 128] = wa[kx]          # ky01 row-pairs
    out[:, 384:512] = wb[0]                                # ky2 col-pair
    out[0:64, 512:640] = wb[1]                             # ky2 kx2
    return out


def _make_in_maps(x, wq, bq, wk, bk, wv, bv):
    QD, KD = _hdist_channels()
    expwb = np.exp(_wbias().T)          # [kc, q]; exp(NEG) == 0 masks exactly
    wq_s = wq * SCALE
    w2 = np.zeros((3, 128, 128), dtype=np.float32)
    v2 = np.zeros((3, 128, 128), dtype=np.float32)
    for kx in range(3):
        w2[kx, 0:64, 0:64] = wq_s[:, :, 0, kx].T
        w2[kx, 0:64, 64:128] = wk[:, :, 0, kx].T
        w2[kx, 64:128, 0:64] = wq_s[:, :, 1, kx].T
        w2[kx, 64:128, 64:128] = wk[:, :, 1, kx].T
        v2[kx, 0:64, :] = wv[:, :, 0, kx].T
        v2[kx, 64:128, :] = wv[:, :, 1, kx].T
    # ky2: kx0+kx1 stacked as a 128-deep column-pair tap, kx2 64-deep
    wqk1p = np.zeros((128, 128), dtype=np.float32)
    wv1p = np.zeros((128, 128), dtype=np.float32)
    for half, kx in ((0, 0), (1, 1)):
        sl = slice(64 * half, 64 * half + 64)
        wqk1p[sl, 0:64] = wq_s[:, :, 2, kx].T
        wqk1p[sl, 64:128] = wk[:, :, 2, kx].T
        wv1p[sl, :] = wv[:, :, 2, kx].T
    wqk1s = np.concatenate([wq_s[:, :, 2, 2].T, wk[:, :, 2, 2].T], axis=1)
    wv1s = wv[:, :, 2, 2].T
    wqk_p = _pack_layer(w2, (wqk1p, wqk1s))
    wv_p = _pack_layer(v2, (wv1p, wv1s))
    qvbias = np.stack(
        [np.concatenate([bq * SCALE, bk]), bv], axis=1).astype(np.float32)

    in_maps = []
    for core in range(NCORES):
        b, slab = core // NH, core % NH
        h0 = slab * HS
        xsl = np.zeros((64, XR, XC), dtype=np.float32)
        r_lo, r_hi = h0 - 7, h0 - 7 + XR  # image rows of x slab
        src_lo, src_hi = max(0, r_lo), min(H, r_hi)
        xsl[:, src_lo - r_lo: src_hi - r_lo, 1:97] = x[b, :, src_lo:src_hi, :]
        qdf = np.repeat(QD[:, h0:h0 + HS, None], 96, axis=2).reshape(NDIST, -1)
        kdf = np.zeros((NDIST, KV, 96), dtype=np.float32)
        for r in range(KV):
            img = h0 - QOFF + r
            if 0 <= img < H:
                kdf[:, r, :] = KD[:, img, None]
        in_maps.append({
            'xs': xsl.astype(NPBF16),
            'wqk': wqk_p.astype(NPBF16),
            'wv': wv_p.astype(NPBF16),
            'qvb': qvbias,
            'qd': np.ascontiguousarray(qdf).astype(NPBF16),
            'kd': np.ascontiguousarray(kdf.reshape(NDIST, -1)).astype(NPBF16),
            'ewb': np.ascontiguousarray(expwb).astype(NPBF16),
        })
    return in_maps


def kernel(x, wq, bq, wk, bk, wv, bv):
    x = np.asarray(x, dtype=np.float32)
    wq = np.asarray(wq, dtype=np.float32)
    wk = np.asarray(wk, dtype=np.float32)
    wv = np.asarray(wv, dtype=np.float32)
    bq = np.asarray(bq, dtype=np.float32)
    bk = np.asarray(bk, dtype=np.float32)
    bv = np.asarray(bv, dtype=np.float32)

    nc = _build_program()
    in_maps = _make_in_maps(x=x, wq=wq, bq=bq, wk=wk, bk=bk, wv=wv, bv=bv)

    from concourse.bass_utils import run_bass_kernel_spmd
    res = run_bass_kernel_spmd(nc, in_maps, core_ids=list(range(NCORES)))
    global LAST_RESULT
    LAST_RESULT = res

    out = np.zeros((B, H, W, CO), dtype=np.float32)
    for core in range(NCORES):
        b, slab = core // NH, core % NH
        raw = res.results[core]['o']          # (96, HS, 129): [num | den]
        raw = np.transpose(raw, (1, 0, 2))
        out[b, slab * HS:(slab + 1) * HS] = raw[..., 0:128] / raw[..., 128:129]

    border = _host_border(x, wq, bq, wk, bk, wv, bv)
    for h, val in border.items():
        out[:, h] = val.astype(np.float32)
    return out


# revision 40
# speedup vs baseline: 1.1228x; 1.1228x over previous
"""ConvNAT (conv QKV + 2D dilated neighborhood attention) on 8 trn2 cores.

Sharding: core = (batch b, H-slab of 24 rows).  Each core computes conv
q/k/v for the 36 kv rows its attention actually reads (image rows
h0-6 .. h0+29) and the attention for its 24 output rows.  H-border rows
(h<6, h>=90), whose NATTEN windows are clamped, are computed on the host
and overwrite the device output.

v2 notes (PE p-state aware schedule):
  * x is loaded as 6 element-shifted flat copies of the same contiguous
    per-partition HBM run (shifts 0/98/1/99 give all row/col tap
    alignments) -- every DMA is one big descriptor per partition.
  * conv is emitted group-major so each psum bank recycles early
    (bufs=4) and the PE stream has no intra-wave stalls.
  * the logits prologue (key rows 0..12) is interleaved INTO the v-conv
    phase so the PE never idles across the conv->attention transition;
    idle gaps reset the tensor engine's DVFS ramp (0.42 -> 0.83 ns/col).
  * W-position bias+mask is applied multiplicatively AFTER exp
    (exp(L+wb) == exp(L)*exp(wb), exp(NEG)=0 masks exactly): ACT does
    exp directly out of PSUM, DVE does a cheap bf16 2X/4X multiply, and
    the psO evacuation moves to Pool.  Per-step engine loads stay under
    the PE's full-speed step time, keeping the PE gapless (and ramped).

Attention (transposed-logits form), unchanged math:
  L^T[kc,(j,q)] = K_ext_r^T Q_ext ; Q/K_ext = [conv ch | 7 h-dist ch].
  px = exp(L^T) * exp(wb)  (wb = scale*pe_w.pe_w + NEG mask, host table)
  psum[96q,129] = sum_i px_i^T^T vt_aug[r_i], col 128 = softmax denom,
  Pool multiplies by 1/den during psum evacuation.
"""
import os
import re
import sys

sys.path.insert(0, '/opt/trn_rl_repo')

import numpy as np
import ml_dtypes

import concourse.bass as bass
from concourse import mybir
from concourse.tile import TileContext
from concourse.masks import make_identity
from bass_rust import ScopedClock, VectorClock

F32 = mybir.dt.float32
BF16 = mybir.dt.bfloat16
NPBF16 = ml_dtypes.bfloat16

B, CIN, H, W = 2, 64, 96, 96
CI, CO = 64, 128
KS, DIL = 7, 2
SCALE = float(CI * 2) ** -0.5  # Cqk = 128 after pe concat
HS = 24          # rows per core
NH = 4           # h-slabs
NCORES = 8
NDIST = 7        # h-distance channels
CH = 64 + NDIST  # contraction channels
KV = 36          # kv rows per core (24 + 6 halo each side; row r = img h0-6+r)
XR = 38          # x slab rows (KV + conv halo; row t = img h0-7+t)
XC = 98          # x slab cols (96 + pad)
QOFF = 6         # q row j = kv row j+6
NEG = -30000.0
RING = 23        # px ring depth (av(j) reads key rows j..j+12; logits runs ~21 ahead)
AVW = 130        # AV moving width: 128 v + ones col + pad (even for bf16)
VTW = 136        # vt row stride (128 v + 1 ones + pad)

GRP = ((0, 5), (5, 10), (10, 15), (15, 20),
       (20, 24), (24, 28), (28, 32), (32, 36))
# logits prologue rows emitted after each v-conv group (front-loaded so the
# exps of the last rows clear the ACT queue before the bridge)
PROLOG = ((0, 1, 2), (3, 4, 5), (6, 7), (8, 9), (10, 11), (12, 13), (14,), (15,))
AHEAD = 18                     # steady pair jp: logits(2jp+AHEAD+2, +3)

# ---------------------------------------------------------------- compat ---
MAX_WAITS = 1


def _patched_drain(self, tick_clock, wait_clock):
    nc = self.nc
    ticks = [int(v) for v in re.findall(r'\d+', repr(tick_clock.global_clock))]
    for i in range(0, len(ticks), MAX_WAITS):
        chunk = [0] * len(ticks)
        chunk[i:i + MAX_WAITS] = ticks[i:i + MAX_WAITS]
        if any(chunk):
            probe = nc.sync.nop()
            wait_clock.add_sem_waits(probe.ins, ScopedClock({None: VectorClock(chunk)}))
    nc.sync.drain()
    nc.all_engine_barrier()
    popped = nc._tile_sem_poison_stack.pop()
    assert popped is self._sem_poison
    nc.clear_and_free_semaphores(list(self.sems.allocated().values()))
    nc.all_engine_barrier()


TileContext._drain_and_barrier = _patched_drain


def _split_excess_waits(nc, max_waits=MAX_WAITS):
    n_split = 0
    for fn in nc.m.functions:
        for bb in fn.blocks:
            out = []
            changed = False
            for inst in bb.instructions:
                si = inst.sync_info
                waits = list(si.on_wait) if si and si.on_wait else []
                if len(waits) > max_waits:
                    extra = waits[:-max_waits]
                    for j in range(0, len(extra), max_waits):
                        nop = mybir.InstNoOp(name=f"{inst.name}-ws{j}", ins=[], outs=[])
                        nop.engine = inst.engine
                        nop.sync_info = mybir.SyncInfo(
                            on_wait=extra[j:j + max_waits], on_update=[])
                        out.append(nop)
                    si.on_wait = waits[-max_waits:]
                    changed = True
                    n_split += 1
                out.append(inst)
            if changed:
                bb.instructions = out
    return n_split


# ------------------------------------------------------------- host math ---
def _sincos(length, dim):
    half = dim // 2
    inv_freq = 1.0 / (10000.0 ** (np.arange(half, dtype=np.float64) * 2.0 / dim))
    ang = np.arange(length, dtype=np.float64)[:, None] * inv_freq[None, :]
    return np.concatenate([np.sin(ang), np.cos(ang)], axis=-1)  # (L, dim)


def _na_indices(L, K, D):
    i = np.arange(L)
    g = i % D
    r = i // D
    Lg = (L - g + D - 1) // D
    start = np.clip(r - K // 2, 0, Lg - K)
    return g[:, None] + (start[:, None] + np.arange(K)[None, :]) * D  # (L, K)


def _hdist_channels():
    """QD,KD (NDIST, 96): sum_m QD[m,h]*KD[m,h'] == SCALE*pe_h[h].pe_h[h']
    exactly for even |h-h'| <= 6."""
    pe = _sincos(H, 32)           # (96, 32)
    inv_freq = 1.0 / (10000.0 ** (np.arange(16, dtype=np.float64) * 2.0 / 32))
    dv = np.array([0., 2., 4., 6.])
    g = SCALE * np.cos(dv[:, None] * inv_freq[None, :]).sum(1)  # exact pe.pe(d)
    th = np.arange(4, dtype=np.float64) * (np.pi / 6.0)
    M = np.cos(dv[:, None] * th[None, :])                        # (4, 4)
    b = np.linalg.solve(M, g)
    hh = np.arange(H, dtype=np.float64)
    QD = np.zeros((NDIST, H))
    KD = np.zeros((NDIST, H))
    QD[0] = b[0]
    KD[0] = 1.0
    for m in range(1, 4):
        QD[2 * m - 1] = b[m] * np.cos(th[m] * hh)
        QD[2 * m] = b[m] * np.sin(th[m] * hh)
        KD[2 * m - 1] = np.cos(th[m] * hh)
        KD[2 * m] = np.sin(th[m] * hh)
    # verify
    got = QD.T @ KD
    ref = SCALE * (pe @ pe.T)
    for dd in (-6, -4, -2, 0, 2, 4, 6):
        idx = np.arange(max(0, -dd), min(H, H - dd))
        err = np.abs(got[idx, idx + dd] - ref[idx, idx + dd]).max()
        assert err < 1e-6, (dd, err)
    return QD.astype(np.float32), KD.astype(np.float32)


def _wbias():
    pe = _sincos(W, 32)
    idx_w = _na_indices(W, KS, DIL)   # (96, 7)
    wb = np.full((W, W), NEG, dtype=np.float64)
    dot = SCALE * (pe @ pe.T)
    for w in range(W):
        wb[w, idx_w[w]] = dot[w, idx_w[w]]
    return wb


def _conv_np(x, w, bias, rows):
    """NCHW 3x3 pad-1 conv evaluated at `rows` -> (B, len(rows), 96, Cout)."""
    Bn, Cin, Hn, Wn = x.shape
    xp = np.zeros((Bn, Cin, Hn + 2, Wn + 2), dtype=np.float64)
    xp[:, :, 1:-1, 1:-1] = x
    rows = np.asarray(rows)
    acc = np.zeros((Bn, len(rows), Wn, w.shape[0]), dtype=np.float64)
    for ky in range(3):
        for kx in range(3):
            xs = xp[:, :, rows + ky, :][:, :, :, kx:kx + Wn]  # (B,C,R,W)
            acc += np.einsum('bcrw,oc->brwo', xs, w[:, :, ky, kx].astype(np.float64))
    return acc + bias[None, None, None, :].astype(np.float64)


def _host_border(x, wq, bq, wk, bk, wv, bv):
    """Reference computation for the clamped border rows. -> dict h -> (B,96,128)."""
    border_h = list(range(0, 6)) + list(range(90, 96))
    kv_rows = sorted(set(np.concatenate([_na_indices(H, KS, DIL)[h] for h in border_h])))
    kv_rows = np.asarray(kv_rows)
    q_c = _conv_np(x, wq, bq, np.asarray(border_h))     # (B, 12, 96, 64)
    k_c = _conv_np(x, wk, bk, kv_rows)                  # (B, R, 96, 64)
    v_c = _conv_np(x, wv, bv, kv_rows)                  # (B, R, 96, 128)
    kv_pos = {r: i for i, r in enumerate(kv_rows)}
    pe_h = _sincos(H, 32)
    pe_w = _sincos(W, 32)
    idx_h = _na_indices(H, KS, DIL)
    idx_w = _na_indices(W, KS, DIL)
    out = {}
    for bi, h in enumerate(border_h):
        pe_q = np.concatenate([np.repeat(pe_h[h][None], W, 0), pe_w], axis=1)  # (96,64)
        q = np.concatenate([q_c[:, bi], np.repeat(pe_q[None], B, 0)], axis=2)  # (B,96,128)
        rows = [kv_pos[r] for r in idx_h[h]]
        kk = k_c[:, rows]                                   # (B,7,96,64)
        vv = v_c[:, rows]                                   # (B,7,96,128)
        pe_k = np.concatenate(
            [np.repeat(pe_h[idx_h[h]][:, None, :], W, 1),
             np.repeat(pe_w[None], KS, 0)], axis=2)         # (7,96,64)
        kk = np.concatenate([kk, np.repeat(pe_k[None], B, 0)], axis=3)  # (B,7,96,128)
        kn = kk[:, :, idx_w]                                # (B,7,96,7,128)
        vn = vv[:, :, idx_w]
        logits = SCALE * np.einsum('bwc,biwjc->bwij', q, kn)   # (B,96,7,7)
        m = logits.reshape(B, W, -1).max(-1)
        p = np.exp(logits - m[:, :, None, None])
        p /= p.reshape(B, W, -1).sum(-1)[:, :, None, None]
        out[h] = np.einsum('bwij,biwjc->bwc', p, vn)        # (B,96,128)
    return out


def _users(r):
    """Query rows j (0..23) attending key row r: j = r-2i, i=0..6."""
    j0 = r - 12 if r >= 12 else r % 2
    j1 = min(r, 23)
    if j0 > j1:
        return j0, 0
    return j0, (j1 - j0) // 2 + 1


# ------------------------------------------------------------ bass build ---
_CACHE = {}


def _build_program():
    if 'nc' in _CACHE:
        return _CACHE['nc']
    nc = bass.Bass('TRN2')
    xs = nc.dram_tensor('xs', (64, XR, XC), BF16, kind='ExternalInput')
    # weights packed into one tensor per conv layer (HWDGE serializes DMAs
    # per queue, so fewer/bigger transfers win):
    #   cols 0:384 = ky01 taps (3x128 contract-pairs), 384:512 = ky2 colpair,
    #   rows 0:64 cols 512:640 = ky2 kx2 (64-contract)
    wqk = nc.dram_tensor('wqk', (128, 640), BF16, kind='ExternalInput')
    wv = nc.dram_tensor('wv', (128, 640), BF16, kind='ExternalInput')
    qvb = nc.dram_tensor('qvb', (128, 2), F32, kind='ExternalInput')
    qd = nc.dram_tensor('qd', (NDIST, HS * 96), BF16, kind='ExternalInput')
    kd = nc.dram_tensor('kd', (NDIST, KV * 96), BF16, kind='ExternalInput')
    ewb = nc.dram_tensor('ewb', (96, 96), BF16, kind='ExternalInput')
    # (96, HS, 129): w-major so a 2-row store is one descriptor/partition;
    # col 128 carries the softmax denominator; host transposes + divides
    o = nc.dram_tensor('o', (96, HS, 129), F32, kind='ExternalOutput')

    with TileContext(nc) as tc:
        with tc.tile_pool(name='persist', bufs=1) as pp:
            # ---- x: three element-shifted flat HBM streams (pitch XC):
            #   x2  bot = shift 0 (row t, col c)   top = shift 98 (row t+1)
            #   xw  top = shift 1 (row t, col c+1)
            # derived on otherwise-idle engines:
            #   x2b[0:128] = x2 cols 1:97  (DVE col-shift copy, both halves)
            #   xw[0:64]   = x2 bot cols 0:96  (ACT copy; Pool copies are 3x
            #                slower and SWDGE triggers hog the Pool queue)
            # HWDGE serializes DMAs per queue (~1.7us each), so x rides on
            # all three queues: x2 bot on sync, x2 top on scalar, xw top on
            # gpsimd, chunked (0,7)/(7,22)/(22,38) to bound the first-tap
            # latency while keeping the queue depth small.
            x2 = pp.tile([128, XR, XC], BF16)
            x2b = pp.tile([128, XR, 96], BF16)
            xw = pp.tile([128, XR, XC], BF16)
            xs_ap = xs[:]

            def flat_dma(eng, tile, p0, a, b, shift):
                off = a * XC + shift
                n = min((b - a) * XC, XR * XC - off)
                if n <= 0:
                    return
                src = bass.AP(tensor=xs_ap.tensor, offset=xs_ap.offset + off,
                              ap=[xs_ap.ap[0], [1, n]])
                d0 = tile[p0:p0 + 64, a:b, :]
                dst = bass.AP(tensor=d0.tensor, offset=d0.offset,
                              ap=[d0.ap[0], [1, n]])
                eng.dma_start(out=dst, in_=src)

            def xchunk(a, b):
                flat_dma(nc.sync, x2, 0, a, b, 0)
                flat_dma(nc.scalar, x2, 64, a, b, XC)
                flat_dma(nc.gpsimd, xw, 64, a, b, 1)
                b37 = min(b, XR - 1)
                nc.vector.tensor_copy(out=x2b[:, a:b37, :],
                                      in_=x2[:, a:b37, 1:97])
                if b > b37:
                    nc.vector.tensor_copy(out=x2b[0:64, b37:b, :],
                                          in_=x2[0:64, b37:b, 1:97])
                nc.scalar.copy(out=xw[0:64, a:b, 0:96],
                               in_=x2[0:64, a:b, 0:96])

            xchunk(0, 7)
            # qk weights: one packed DMA on the gpsimd queue (keeps the sync
            # queue clear for the x2 chunks the first conv groups gate on)
            w_all = pp.tile([128, 640], BF16)
            nc.gpsimd.dma_start(out=w_all, in_=wqk[:])
            xchunk(7, 22)
            xchunk(22, XR)
            qvbias = pp.tile([128, 2], F32)
            nc.scalar.dma_start(out=qvbias, in_=qvb[:])

            # ---- v weights + tables (needed later; queue behind x)
            wv_all = pp.tile([128, 640], BF16)
            nc.scalar.dma_start(out=wv_all, in_=wv[:])
            ewb_sb = pp.tile([96, 96], BF16)
            nc.scalar.dma_start(out=ewb_sb, in_=ewb[:])
            ident = pp.tile([128, 128], BF16)
            make_identity(nc, ident)

            q_ext = pp.tile([CH, HS * 96], BF16)
            k_ext = pp.tile([CH, KV * 96], BF16)
            vsb = pp.tile([128, KV * 96], BF16)
            vt = pp.tile([96, KV, VTW], BF16)
            nc.sync.dma_start(out=q_ext[64:CH, :], in_=qd[:])
            nc.gpsimd.dma_start(out=k_ext[64:CH, :], in_=kd[:])
            nc.gpsimd.memset(vt[:, :, 128:130], 1.0)

            with tc.tile_pool(name='lps', bufs=2, space='PSUM') as lps, \
                 tc.tile_pool(name='pxr', bufs=3) as pxr, \
                 tc.tile_pool(name='pxp', bufs=RING) as pxp, \
                 tc.tile_pool(name='att', bufs=3) as att:
                pexp = {}
                ewb_ap = ewb_sb[:, :]

                def logits(r):
                    j0, n = _users(r)
                    # rows padded to 128 f32 so 4 rows = one PSUM bank
                    psL = lps.tile([96, 7, 128], F32, tag='L')
                    ke = k_ext[:, r * 96:(r + 1) * 96]
                    for (u0, u1) in ((0, min(n, 4)), (4, n)):
                        if u0 >= u1:
                            continue
                        qa = q_ext[:, (j0 + 2 * u0) * 96:(j0 + 2 * u0 + 1) * 96]
                        rhs = bass.AP(tensor=qa.tensor, offset=qa.offset,
                                      ap=[qa.ap[0], [2 * 96, u1 - u0], qa.ap[1]])
                        nc.tensor.matmul(psL[:, u0:u1, 0:96], ke, rhs,
                                         start=True, stop=True)
                    # exp straight out of psum; W-bias applied multiplicatively
                    # (exp(NEG)=0 masks exactly), bf16 all-SBUF 2X on DVE.
                    pxr_t = pxr.tile([96, 7, 96], BF16, tag='pxr')
                    nc.scalar.activation(out=pxr_t[:, 0:n, :],
                                         in_=psL[:, 0:n, 0:96],
                                         func=mybir.ActivationFunctionType.Exp)
                    px = pxp.tile([96, 7, 96], BF16, tag='px')
                    ewb_b = bass.AP(tensor=ewb_ap.tensor, offset=ewb_ap.offset,
                                    ap=[ewb_ap.ap[0], [0, n], ewb_ap.ap[1]])
                    # all-SBUF bf16 multiply; Pool (otherwise idle) takes the
                    # late rows fully and the tail slots of the early steady
                    # rows, so DVE keeps up with oh/vt evacuations
                    if r >= 24 or r in (14, 15):
                        nc.gpsimd.tensor_tensor(out=px[:, 0:n, :],
                                                in0=pxr_t[:, 0:n, :],
                                                in1=ewb_b,
                                                op=mybir.AluOpType.mult)
                    elif r >= 18 and n > 5:
                        ewb_b5 = bass.AP(tensor=ewb_ap.tensor,
                                         offset=ewb_ap.offset,
                                         ap=[ewb_ap.ap[0], [0, 5],
                                             ewb_ap.ap[1]])
                        ewb_b2 = bass.AP(tensor=ewb_ap.tensor,
                                         offset=ewb_ap.offset,
                                         ap=[ewb_ap.ap[0], [0, n - 5],
                                             ewb_ap.ap[1]])
                        nc.vector.tensor_tensor(out=px[:, 0:5, :],
                                                in0=pxr_t[:, 0:5, :],
                                                in1=ewb_b5,
                                                op=mybir.AluOpType.mult)
                        nc.gpsimd.tensor_tensor(out=px[:, 5:n, :],
                                                in0=pxr_t[:, 5:n, :],
                                                in1=ewb_b2,
                                                op=mybir.AluOpType.mult)
                    else:
                        nc.vector.tensor_tensor(out=px[:, 0:n, :],
                                                in0=pxr_t[:, 0:n, :],
                                                in1=ewb_b,
                                                op=mybir.AluOpType.mult)
                    pexp[r] = px

                # ---------------------------------------------- convolution --
                # group-major: each group's 5 taps back-to-back, psum bank
                # recycles as soon as its evac lands (bufs=4).  The logits
                # prologue (rows 0..12) interleaves into the v layer so the
                # PE stream never breaks at the conv->attention transition.
                with tc.tile_pool(name='cps', bufs=4, space='PSUM') as cps:
                    for which in range(2):  # 0 = qk, 1 = v
                        wt = w_all if which == 0 else wv_all
                        for g in range(8):
                            r0, r1 = GRP[g]
                            sz = r1 - r0
                            ps = cps.tile([128, 5, 96], F32, tag='c',
                                          name=f'c{which}_{g}')
                            nc.tensor.matmul(ps[:, 0:sz, :], wt[:, 0:128],
                                             x2[:, r0:r1, 0:96],
                                             start=True, stop=False)
                            nc.tensor.matmul(ps[:, 0:sz, :], wt[:, 128:256],
                                             x2b[:, r0:r1, 0:96],
                                             start=False, stop=False)
                            nc.tensor.matmul(ps[:, 0:sz, :], wt[:, 256:384],
                                             x2[:, r0:r1, 2:XC],
                                             start=False, stop=False)
                            nc.tensor.matmul(ps[:, 0:sz, :], wt[:, 384:512],
                                             xw[:, r0 + 2:r1 + 2, 0:96],
                                             start=False, stop=False)
                            nc.tensor.matmul(ps[:, 0:sz, :],
                                             wt[0:64, 512:640],
                                             x2[0:64, r0 + 2:r1 + 2, 2:XC],
                                             start=False, stop=True)
                            if which == 0:
                                # k rows: all; q rows: kv rows 6..29 only
                                nc.vector.tensor_scalar(
                                    out=k_ext[0:64, r0 * 96:r1 * 96],
                                    in0=ps[64:128, 0:sz, :],
                                    scalar1=qvbias[64:128, 0:1],
                                    scalar2=None, op0=mybir.AluOpType.add)
                                qa = max(r0, QOFF) - r0
                                qb = min(r1, QOFF + HS) - r0
                                if qa < qb:
                                    nc.scalar.activation(
                                        out=q_ext[0:64, (r0 + qa - QOFF) * 96:
                                                  (r0 + qb - QOFF) * 96],
                                        in_=ps[0:64, qa:qb, :],
                                        func=mybir.ActivationFunctionType.Identity,
                                        bias=qvbias[0:64, 0:1])
                            else:
                                if g % 2 == 0:
                                    nc.scalar.activation(
                                        out=vsb[:, r0 * 96:r1 * 96],
                                        in_=ps[:, 0:sz, :],
                                        func=mybir.ActivationFunctionType.Identity,
                                        bias=qvbias[:, 1:2])
                                else:
                                    nc.vector.tensor_scalar(
                                        out=vsb[:, r0 * 96:r1 * 96],
                                        in0=ps[:, 0:sz, :],
                                        scalar1=qvbias[:, 1:2],
                                        scalar2=None, op0=mybir.AluOpType.add)
                                for r in PROLOG[g]:
                                    logits(r)

                # ------------------------------- V^T + attention steady ----
                with tc.tile_pool(name='tps', bufs=1, space='PSUM') as tps, \
                     tc.tile_pool(name='ops', bufs=3, space='PSUM') as ops:
                    # NOTE: a start=True matmul marks its whole 2KB psum bank
                    # pending-zero, so two 520B psO slots share one bank: the
                    # second accumulation group opens with start=False and its
                    # first write lands on pending-zero bytes (write mode).
                    dve_cp = nc.vector.tensor_copy
                    act_cp = nc.scalar.copy
                    vt_pend = []

                    def vtrans(grp, enga=None, engb=None):
                        pst = tps.tile([96, 4, 128], BF16, tag='t')
                        for rr in range(4):
                            r = grp * 4 + rr
                            nc.tensor.transpose(
                                pst[:, rr, :], vsb[:, r * 96:(r + 1) * 96], ident)
                        if enga is None:
                            vt_pend.extend((pst, grp, rr) for rr in range(4))
                        else:
                            enga(out=vt[:, grp * 4:grp * 4 + 2, 0:128],
                                 in_=pst[:, 0:2, :])
                            engb(out=vt[:, grp * 4 + 2:grp * 4 + 4, 0:128],
                                 in_=pst[:, 2:4, :])

                    def av_pair(j0):
                        psO = ops.tile([96, 2, 136], F32, tag='O')
                        for s in range(2):
                            j = j0 + s
                            for i in range(KS):
                                r = j + 2 * i
                                ju, _ = _users(r)
                                slot = (j - ju) // 2
                                nc.tensor.matmul(psO[:, s, 0:AVW],
                                                 pexp[r][:, slot, :],
                                                 vt[:, r, 0:AVW],
                                                 start=(s == 0 and i == 0),
                                                 stop=(i == KS - 1),
                                                 skip_group_check=(s == 1))
                        # raw evac incl. denominator col; host divides
                        oh = att.tile([96, 2, 129], F32, tag='oh')
                        nc.vector.tensor_copy(out=oh[:], in_=psO[:, 0:2, 0:129])
                        nc.sync.dma_start(out=o[:, j0:j0 + 2, :], in_=oh[:])

                    # bridge: v^T transposes while the conv psum pool drains;
                    # logits interleave to cover the tps=1 evac latency.
                    # DVE takes most evac halves -- ACT must keep exps flowing
                    # or the first steady logits stall (p-state demotion).
                    for grp in range(6):
                        vtrans(grp, dve_cp,
                               act_cp if grp % 2 else dve_cp)
                        if grp < 4:
                            logits(AHEAD - 2 + grp)        # rows 16..19
                    for jp in range(HS // 2):
                        j0 = 2 * jp
                        for r in (j0 + AHEAD + 2, j0 + AHEAD + 3):
                            if r < KV:
                                logits(r)
                        if jp in (1, 3, 5):
                            vtrans(6 + jp // 2)     # evacs drain below
                        for _ in range(2):
                            if vt_pend:
                                pst, grp, rr = vt_pend.pop(0)
                                dve_cp(out=vt[:, grp * 4 + rr, 0:128],
                                       in_=pst[:, rr, :])
                        av_pair(j0)

    _split_excess_waits(nc)
    _CACHE['nc'] = nc
    return nc


# ---------------------------------------------------------------- kernel ---
def _pack_layer(wa, wb):
    """Pack a conv layer's taps into the (128, 640) device layout.
    wa/wb: (64-or-128 out-ch halves already merged) -> see caller."""
    out = np.zeros((128, 640), dtype=np.float32)
    for kx in range(3):
        out[:, kx * 128:(kx + 1) * 128] = wa[kx]          # ky01 row-pairs
    out[:, 384:512] = wb[0]                                # ky2 col-pair
    out[0:64, 512:640] = wb[1]                             # ky2 kx2
    return out


def _make_in_maps(x, wq, bq, wk, bk, wv, bv):
    QD, KD = _hdist_channels()
    expwb = np.exp(_wbias().T)          # [kc, q]; exp(NEG) == 0 masks exactly
    wq_s = wq * SCALE
    w2 = np.zeros((3, 128, 128), dtype=np.float32)
    v2 = np.zeros((3, 128, 128), dtype=np.float32)
    for kx in range(3):
        w2[kx, 0:64, 0:64] = wq_s[:, :, 0, kx].T
        w2[kx, 0:64, 64:128] = wk[:, :, 0, kx].T
        w2[kx, 64:128, 0:64] = wq_s[:, :, 1, kx].T
        w2[kx, 64:128, 64:128] = wk[:, :, 1, kx].T
        v2[kx, 0:64, :] = wv[:, :, 0, kx].T
        v2[kx, 64:128, :] = wv[:, :, 1, kx].T
    # ky2: kx0+kx1 stacked as a 128-deep column-pair tap, kx2 64-deep
    wqk1p = np.zeros((128, 128), dtype=np.float32)
    wv1p = np.zeros((128, 128), dtype=np.float32)
    for half, kx in ((0, 0), (1, 1)):
        sl = slice(64 * half, 64 * half + 64)
        wqk1p[sl, 0:64] = wq_s[:, :, 2, kx].T
        wqk1p[sl, 64:128] = wk[:, :, 2, kx].T
        wv1p[sl, :] = wv[:, :, 2, kx].T
    wqk1s = np.concatenate([wq_s[:, :, 2, 2].T, wk[:, :, 2, 2].T], axis=1)
    wv1s = wv[:, :, 2, 2].T
    wqk_p = _pack_layer(w2, (wqk1p, wqk1s))
    wv_p = _pack_layer(v2, (wv1p, wv1s))
    qvbias = np.stack(
        [np.concatenate([bq * SCALE, bk]), bv], axis=1).astype(np.float32)

    in_maps = []
    for core in range(NCORES):
        b, slab = core // NH, core % NH
        h0 = slab * HS
        xsl = np.zeros((64, XR, XC), dtype=np.float32)
        r_lo, r_hi = h0 - 7, h0 - 7 + XR  # image rows of x slab
        src_lo, src_hi = max(0, r_lo), min(H, r_hi)
        xsl[:, src_lo - r_lo: src_hi - r_lo, 1:97] = x[b, :, src_lo:src_hi, :]
        qdf = np.repeat(QD[:, h0:h0 + HS, None], 96, axis=2).reshape(NDIST, -1)
        kdf = np.zeros((NDIST, KV, 96), dtype=np.float32)
        for r in range(KV):
            img = h0 - QOFF + r
            if 0 <= img < H:
                kdf[:, r, :] = KD[:, img, None]
        in_maps.append({
            'xs': xsl.astype(NPBF16),
            'wqk': wqk_p.astype(NPBF16),
            'wv': wv_p.astype(NPBF16),
            'qvb': qvbias,
            'qd': np.ascontiguousarray(qdf).astype(NPBF16),
            'kd': np.ascontiguousarray(kdf.reshape(NDIST, -1)).astype(NPBF16),
            'ewb': np.ascontiguousarray(expwb).astype(NPBF16),
        })
    return in_maps


def kernel(x, wq, bq, wk, bk, wv, bv):
    x = np.asarray(x, dtype=np.float32)
    wq = np.asarray(wq, dtype=np.float32)
    wk = np.asarray(wk, dtype=np.float32)
    wv = np.asarray(wv, dtype=np.float32)
    bq = np.asarray(bq, dtype=np.float32)
    bk = np.asarray(bk, dtype=np.float32)
    bv = np.asarray(bv, dtype=np.float32)

    nc = _build_program()
    in_maps = _make_in_maps(x=x, wq=wq, bq=bq, wk=wk, bk=bk, wv=wv, bv=bv)

    from concourse.bass_utils import run_bass_kernel_spmd
    res = run_bass_kernel_spmd(nc, in_maps, core_ids=list(range(NCORES)))
    global LAST_RESULT
    LAST_RESULT = res

    out = np.zeros((B, H, W, CO), dtype=np.float32)
    for core in range(NCORES):
        b, slab = core // NH, core % NH
        raw = res.results[core]['o']          # (96, HS, 129): [num | den]
        raw = np.transpose(raw, (1, 0, 2))
        out[b, slab * HS:(slab + 1) * HS] = raw[..., 0:128] / raw[..., 128:129]

    border = _host_border(x, wq, bq, wk, bk, wv, bv)
    for h, val in border.items():
        out[:, h] = val.astype(np.float32)
    return out
